# revision 17
# baseline (speedup 1.0000x reference)
"""BipartiteSAGEConv Trainium2 kernel.

Strategy: destination-sharded, zero collectives.
- Host: sort/partition edges by destination across 8 cores (6250 dsts each),
  group per 128-dst tile, split by src half (int16 index limit), pad to
  128-edge chunks (uniform chunk structure across cores so one SPMD program
  serves all 8 cores).
- Device per core: dma_gather (MoE row-gather ucode) pulls per-edge src rows
  HBM->SBUF; scatter-add via one-hot matmul on the TensorEngine accumulates
  [dst,128] sums + counts in PSUM; divide by count; two linear layers + bias
  via PE matmuls; DMA out the [6250,128] shard.
"""

import sys
import types

import numpy as np

N_SRC = 50000
N_DST = 50000
E = 800000
D = 128
OUT = 128
N_CORES = 8
P = 128
DST_PER_CORE = N_DST // N_CORES          # 6250
TILES = (DST_PER_CORE + P - 1) // P      # 49
HALF = 25000                             # int16 index limit split
MAX_ROWS_PER_GATHER = 1024               # SWDGE ring limit (measured)


def _install_ntff_hook():
    try:
        import antenv
        if "antenv.axon_hooks" in sys.modules:
            return
        mod = types.ModuleType("antenv.axon_hooks")
        _h = [None]
        mod.set_axon_ntff_profile_hook = lambda h: _h.__setitem__(0, h)
        mod.get_axon_ntff_profile_hook = lambda: _h[0]
        sys.modules["antenv.axon_hooks"] = mod
        antenv.axon_hooks = mod
        from trn_agent_boot.trn_boot import _ntff_profile_via_ctypes
        mod.set_axon_ntff_profile_hook(
            _ntff_profile_via_ctypes("/opt/axon/libaxon_pjrt.so"))
    except Exception:
        pass


def _prep_core(edge_src, edge_dst, core):
    """Per-core edge structure: for each (tile, half) return the edge lists.

    Returns list over 49 tiles of (src_lo, dstl_lo, src_hi, dstl_hi) where
    src_* are int64 source indices (absolute) and dstl_* are local-dst-in-tile
    ids, both sorted by dst.
    """
    lo = core * DST_PER_CORE
    m = (edge_dst >= lo) & (edge_dst < lo + DST_PER_CORE)
    es = edge_src[m]
    ed = edge_dst[m] - lo
    order = np.argsort(ed, kind="stable")
    es, ed = es[order], ed[order]
    tiles = []
    tile_id = ed >> 7
    bounds = np.searchsorted(tile_id, np.arange(TILES + 1))
    for t in range(TILES):
        a, b = bounds[t], bounds[t + 1]
        s, dl = es[a:b], ed[a:b] - t * P
        is_lo = s < HALF
        tiles.append((s[is_lo], dl[is_lo], s[~is_lo] - HALF, dl[~is_lo]))
    return tiles


def _pad_chunks(src, dstl, n_chunks):
    """Pad to n_chunks*128 edges; pad idx=0 (valid row), dstl=-1 (no one-hot)."""
    n = n_chunks * P
    s = np.zeros(n, np.int16)
    d = np.full(n, -1.0, np.float32)
    s[:len(src)] = src.astype(np.int16)
    d[:len(dstl)] = dstl.astype(np.float32)
    return s, d


def _wrap_idx(idx_flat):
    """dma_gather wrapped index layout: index j at partition j%16, col j//16,
    replicated across the 8 gpsimd cores (partition groups of 16)."""
    n = len(idx_flat)
    w = idx_flat.reshape(n // 16, 16).T          # [16, n/16]
    return np.tile(w, (8, 1))                    # [128, n/16]


def build_and_run(x_src, x_dst, edge_src, edge_dst, W_neigh, b_neigh,
                  W_self, b_self):
    _install_ntff_hook()
    from concourse import bacc, bass, mybir, tile
    from concourse.bass_utils import run_bass_kernel_spmd
    from concourse.masks import make_identity

    F32 = mybir.dt.float32
    import os as _os
    use_f16 = _os.environ.get("BSAGE_F32", "0") != "1"
    DTAB = mybir.dt.float16 if use_f16 else F32
    np_tab = np.float16 if use_f16 else np.float32

    # ---------- host-side sharding / layout ----------
    per_core_tiles = [_prep_core(edge_src, edge_dst, c) for c in range(N_CORES)]

    # uniform chunk counts across cores (SPMD: one program, 8 data sets)
    KL = [max(max(1, -(-len(per_core_tiles[c][t][0]) // P))
              for c in range(N_CORES)) for t in range(TILES)]
    KH = [max(max(1, -(-len(per_core_tiles[c][t][2]) // P))
              for c in range(N_CORES)) for t in range(TILES)]
    KE = [KL[t] + KH[t] for t in range(TILES)]
    NCH = sum(KE)                                 # total chunks per core
    KEMAX = max(KE)

    # quad grouping: 4 tiles share one g buffer; chunk layout within a quad:
    # [lo(t0)|lo(t1)|lo(t2)|lo(t3)|hi(t0)|hi(t1)|hi(t2)|hi(t3)]
    _sizes = [8] * ((TILES - 9) // 8) + [4, 2, 2, 1]
    _rem = TILES - sum(_sizes)
    _sizes = [8] * (_rem // 8) + ([_rem % 8] if _rem % 8 else []) + _sizes if _rem > 0 else _sizes
    QUADS = []
    _q = 0
    for _s in _sizes:
        QUADS.append(list(range(_q, _q + _s)))
        _q += _s
    assert _q == TILES, (_q, TILES, _sizes)
    # chunk offset of each (tile, half) within its quad buffer
    chunk_off = {}
    quad_chunks = []
    for qi, qts in enumerate(QUADS):
        off = 0
        for t in qts:
            chunk_off[(t, 0)] = off
            off += KL[t]
        for t in qts:
            chunk_off[(t, 1)] = off
            off += KH[t]
        quad_chunks.append(off)
    KQMAX = max(quad_chunks)

    # gather plan: per quad per half, one contiguous chunk span covering the
    # member tiles' chunks, split into <=8-chunk (1024-row) instructions.
    # gathers: (quad, half, chunk_off_in_quad, n_chunks, idx_col_base)
    gathers = []
    idx_cols = 0                                  # int16 columns consumed
    for qi, qts in enumerate(QUADS):
        for half in (0, 1):
            span = sum((KL if half == 0 else KH)[t] for t in qts)
            base = chunk_off[(qts[0], half)]
            k_done = 0
            while k_done < span:
                k = min(span - k_done, MAX_ROWS_PER_GATHER // P)
                gathers.append((qi, half, base + k_done, k, idx_cols))
                idx_cols += k * 8
                k_done += k
    IDXCOLS = idx_cols

    # per-core data arrays
    idx_all = np.zeros((N_CORES, P, IDXCOLS), np.int16)
    dstl_all = np.zeros((N_CORES, P, NCH), np.float32)
    cbase = np.concatenate([[0], np.cumsum(KE)])  # chunk col base per tile
    for c in range(N_CORES):
        for t in range(TILES):
            s_lo, d_lo, s_hi, d_hi = per_core_tiles[c][t]
            sl, dl = _pad_chunks(s_lo, d_lo, KL[t])
            sh, dh = _pad_chunks(s_hi, d_hi, KH[t])
            s_cat = np.concatenate([sl, sh])
            d_cat = np.concatenate([dl, dh])
            # dstl layout: [128, NCH]; slot p of chunk k = edge k*128+p
            dstl_all[c][:, cbase[t]:cbase[t + 1]] = (
                d_cat.reshape(KE[t], P).T)
        # per-quad padded source-index streams (chunk layout order)
        quad_src = []
        for qi, qts in enumerate(QUADS):
            parts = []
            for t in qts:
                s_lo, d_lo, _, _ = per_core_tiles[c][t]
                parts.append(_pad_chunks(s_lo, d_lo, KL[t])[0])
            for t in qts:
                _, _, s_hi, d_hi = per_core_tiles[c][t]
                parts.append(_pad_chunks(s_hi, d_hi, KH[t])[0])
            quad_src.append(np.concatenate(parts))
        for (qi, half, off, k, colb) in gathers:
            rows = quad_src[qi][off * P:(off + k) * P]
            idx_all[c][:, colb:colb + k * 8] = _wrap_idx(rows)

    x_lo = np.ascontiguousarray(x_src[:HALF]).astype(np_tab)
    x_hi = np.ascontiguousarray(x_src[HALF:]).astype(np_tab)
    xdstT = np.zeros((N_CORES, P, TILES * P), np.float32)
    for c in range(N_CORES):
        shard = x_dst[c * DST_PER_CORE:(c + 1) * DST_PER_CORE]  # [6250,128]
        xdstT[c][:, :DST_PER_CORE] = shard.T
    iota = np.tile(np.arange(P, dtype=np.float32), (P, 1))
    wn = W_neigh.astype(np.float32)
    ws = W_self.astype(np.float32)
    bsum = (b_neigh + b_self).astype(np.float32)[None, :]  # [1,128]

    # ---------- device program ----------
    nc = bacc.Bacc("TRN2", target_bir_lowering=False, debug=False,
                   num_devices=N_CORES, num_swdge_queues=4)
    xlo_d = nc.dram_tensor("xlo", [HALF, D], DTAB, kind="ExternalInput").ap()
    xhi_d = nc.dram_tensor("xhi", [HALF, D], DTAB, kind="ExternalInput").ap()
    idx_d = nc.dram_tensor("idx", [P, IDXCOLS], mybir.dt.int16,
                           kind="ExternalInput").ap()
    dstl_d = nc.dram_tensor("dstl", [P, NCH], F32, kind="ExternalInput").ap()
    xdstT_d = nc.dram_tensor("xdstT", [P, TILES * P], F32,
                             kind="ExternalInput").ap()
    iota_d = nc.dram_tensor("iota", [P, P], F32, kind="ExternalInput").ap()
    wn_d = nc.dram_tensor("wn", [D, OUT], F32, kind="ExternalInput").ap()
    ws_d = nc.dram_tensor("ws", [D, OUT], F32, kind="ExternalInput").ap()
    bsum_d = nc.dram_tensor("bsum", [1, OUT], F32, kind="ExternalInput").ap()
    out_d = nc.dram_tensor("out", [DST_PER_CORE, OUT], F32,
                           kind="ExternalOutput").ap()

    with tile.TileContext(nc) as tc:
        with (
            tc.tile_pool(name="const", bufs=1) as cpool,
            tc.tile_pool(name="work", bufs=4) as wpool,
            tc.tile_pool(name="psum", bufs=2, space="PSUM") as ppool,
        ):
            idx_sb = cpool.tile([P, IDXCOLS], mybir.dt.int16)
            dstl_sb = cpool.tile([P, NCH], F32)
            xdstT_sb = cpool.tile([P, TILES * P], F32)
            iota_sb = cpool.tile([P, P], F32)
            wn_sb = cpool.tile([D, OUT], F32)
            ws_sb = cpool.tile([D, OUT], F32)
            bsum_sb = cpool.tile([1, OUT], F32)
            ones_sb = cpool.tile([P, 1], DTAB)
            ones_row = cpool.tile([1, P], F32)
            ident_sb = cpool.tile([P, P], F32)
            cols_g0 = max(g[4] + g[3] * 8 for g in gathers if g[0] == 0)
            nc.sync.dma_start(out=idx_sb[:, :cols_g0], in_=idx_d[:, :cols_g0])
            nc.sync.dma_start(out=idx_sb[:, cols_g0:], in_=idx_d[:, cols_g0:])
            nc.sync.dma_start(out=dstl_sb[:], in_=dstl_d[:])
            nc.sync.dma_start(out=iota_sb[:], in_=iota_d[:])
            nc.scalar.dma_start(out=xdstT_sb[:], in_=xdstT_d[:])
            nc.scalar.dma_start(out=wn_sb[:], in_=wn_d[:])
            nc.scalar.dma_start(out=ws_sb[:], in_=ws_d[:])
            nc.scalar.dma_start(out=bsum_sb[:], in_=bsum_d[:])
            nc.vector.memset(ones_sb[:], 1.0)
            nc.vector.memset(ones_row[:], 1.0)
            make_identity(nc, ident_sb[:])

            def _emit_tile(t, g_sb):
                ke = KE[t]
                # batched one-hot: oh[p, k*128+j] = (iota[p,j] == dstl[p,cb+k])
                oh_sb = wpool.tile([P, KEMAX * P], DTAB, tag="oh", name=f"oh{t}")
                i_ap = iota_sb[:]
                iota3d = bass.AP(i_ap.tensor, i_ap.offset,
                                 [i_ap.ap[0], [0, ke], [i_ap.ap[1][0], P]])
                d_ap = dstl_sb[:]
                dstl3d = bass.AP(d_ap.tensor, d_ap.offset + int(cbase[t]),
                                 [d_ap.ap[0], [d_ap.ap[1][0], ke], [0, P]])
                oh3d = bass.AP(oh_sb[:].tensor, oh_sb[:].offset,
                               [oh_sb[:].ap[0], [P, ke], [1, P]])
                nc.vector.tensor_tensor(out=oh3d, in0=iota3d, in1=dstl3d,
                                        op=mybir.AluOpType.is_equal)

                ps1 = ppool.tile([P, 132], F32, tag="ps1", name=f"ps1_{t}",
                                 space="PSUM", bufs=3)
                def gchunk(k):
                    if k < KL[t]:
                        return chunk_off[(t, 0)] + k
                    return chunk_off[(t, 1)] + k - KL[t]
                for k in range(ke):
                    gk = gchunk(k)
                    nc.tensor.matmul(
                        out=ps1[:, 0:D],
                        lhsT=oh_sb[:, k * P:(k + 1) * P],
                        rhs=g_sb[:, gk * P:(gk + 1) * P],
                        start=(k == 0), stop=(k == ke - 1))
                for k in range(ke):
                    nc.tensor.matmul(
                        out=ps1[:, D:D + 1],
                        lhsT=oh_sb[:, k * P:(k + 1) * P],
                        rhs=ones_sb[:],
                        start=(k == 0), stop=(k == ke - 1))

                cnt_sb = wpool.tile([P, 1], F32, tag="cnt", name=f"cnt{t}")
                nc.vector.tensor_scalar_max(out=cnt_sb[:], in0=ps1[:, D:D + 1],
                                            scalar1=1.0)
                rcnt_sb = wpool.tile([P, 1], F32, tag="rcnt", name=f"rc{t}")
                nc.vector.reciprocal(out=rcnt_sb[:], in_=cnt_sb[:])
                agg_sb = wpool.tile([P, D], F32, tag="agg", name=f"agg{t}")
                nc.vector.tensor_tensor(out=agg_sb[:], in0=ps1[:, 0:D],
                                        in1=rcnt_sb[:].to_broadcast([P, D]),
                                        op=mybir.AluOpType.mult)
                ps_t = ppool.tile([P, P], F32, tag="pst", name=f"pst{t}",
                                  space="PSUM", bufs=3)
                nc.tensor.transpose(out=ps_t[:], in_=agg_sb[:],
                                    identity=ident_sb[:])
                aggT_sb = wpool.tile([P, D], F32, tag="aggT", name=f"agT{t}")
                nc.vector.tensor_copy(out=aggT_sb[:], in_=ps_t[:])

                ps2 = ppool.tile([P, OUT], F32, tag="ps2", name=f"ps2_{t}",
                                 space="PSUM")
                nc.tensor.matmul(out=ps2[:], lhsT=aggT_sb[:], rhs=wn_sb[:],
                                 start=True, stop=False)
                nc.tensor.matmul(out=ps2[:],
                                 lhsT=xdstT_sb[:, t * P:(t + 1) * P],
                                 rhs=ws_sb[:], start=False, stop=False)
                nc.tensor.matmul(out=ps2[:], lhsT=ones_row[:], rhs=bsum_sb[:],
                                 start=False, stop=True)
                o_sb = wpool.tile([P, OUT], F32, tag="osb", name=f"o{t}")
                nc.scalar.copy(out=o_sb[:], in_=ps2[:])
                rows = min(P, DST_PER_CORE - t * P)
                nc.sync.dma_start(out=out_d[t * P:t * P + rows, :],
                                  in_=o_sb[:rows, :])

            gq = [0]
            g_by_quad = [[] for _ in range(len(QUADS))]
            for g in gathers:
                g_by_quad[g[0]].append(g)
            for qi, qts in enumerate(QUADS):
                g_sb = wpool.tile([P, KQMAX * P], DTAB, tag="g", name=f"g{qi}", bufs=3)
                for (_, half, off, k, colb) in g_by_quad[qi]:
                    t_ap = g_sb[:]
                    out3d = bass.AP(t_ap.tensor, t_ap.offset + off * P,
                                    [t_ap.ap[0], [P, k], [1, P]])
                    nc.gpsimd.dma_gather(
                        out3d,
                        (xlo_d if half == 0 else xhi_d)[:],
                        idx_sb[:, colb:colb + k * 8],
                        k * P,
                        k * P,
                        D,
                        queue_num=(gq[0] % 4),
                    )
                    gq[0] += 1
                for t in qts:
                    _emit_tile(t, g_sb)

    nc.finalize()

    in_maps = [{
        "xlo": x_lo, "xhi": x_hi, "idx": idx_all[c], "dstl": dstl_all[c],
        "xdstT": xdstT[c], "iota": iota, "wn": wn, "ws": ws, "bsum": bsum,
    } for c in range(N_CORES)]

    import os
    trace = os.environ.get("BSAGE_TRACE", "0") == "1"
    res = run_bass_kernel_spmd(nc, in_maps, core_ids=list(range(N_CORES)),
                               trace=trace)
    out = np.concatenate([res.results[c]["out"] for c in range(N_CORES)],
                         axis=0)
    if trace:
        build_and_run.last_exec_ns = res.exec_time_ns
    return out


def kernel(x_src, x_dst, edge_src, edge_dst, num_dst, W_neigh, b_neigh,
           W_self, b_self):
    x_src = np.asarray(x_src, dtype=np.float32)
    x_dst = np.asarray(x_dst, dtype=np.float32)
    edge_src = np.asarray(edge_src).astype(np.int64)
    edge_dst = np.asarray(edge_dst).astype(np.int64)
    W_neigh = np.asarray(W_neigh, dtype=np.float32)
    b_neigh = np.asarray(b_neigh, dtype=np.float32)
    W_self = np.asarray(W_self, dtype=np.float32)
    b_self = np.asarray(b_self, dtype=np.float32)
    return build_and_run(x_src, x_dst, edge_src, edge_dst, W_neigh, b_neigh,
                         W_self, b_self)
